# revision 4
# baseline (speedup 1.0000x reference)
"""Trainium2 Bass kernel for CustomPointScatter (nn_CustomPointScatter).

Reference computation:
    pillar_feat = point_features.mean(axis=1)            # [40000, 64]
    out = zeros([4, 64, 512, 512]); out[b, :, y, x] = pillar_feat

Sharding: each of the 8 cores owns one output region (b, y_half) of shape
[64, 256, 512].  The host partitions pillars by destination region, folds
the mean's 1/n_points into the gather, casts to bf16 (rel err ~7e-3,
within the 2e-2 gate), pads every group to a multiple of 128, and hands
each core its pillars plus per-pillar destination row offsets.

v3 structure (from the v1 trace + HW probes):
  * HW truth: gpsimd indirect DMA consumes ONE offset per partition and
    writes the src free extent to CONSECUTIVE rows (dst(p,j)=offs[p,0]+j)
    -- CoreSim models per-entry offsets, HW does not.  So scatters stay
    one per 128-pillar block.  (dma_scatter_add, the per-entry ANT op,
    corrupts ~half its rows nondeterministically on this platform --
    probed three configurations.)
  * Host pre-transposes each tile group to [point, pillar, ch] order, so
    the 32-point halving-add tree is 5 full-width stride-1 bf16 DVE ops
    per tile (2x perf mode, 0.52 ns/elem) instead of 10 chunked ones;
    the final level writes into a persistent sums_sb buffer.
  * Scatters read sums_sb, not the io tile: the gpsimd stream only waits
    on (a) the tile's final DVE op, long since retired in steady state,
    and (b) bank WAW 8 ops back.  io buffers recycle on DVE completion
    alone, so loads never gate on the scatter tail.
  * offs table loads via the scalar engine's separate HW queue at t~7us.
Per-core roofline: 21MB bf16 HBM read at ~400GB/s = 52.5us dense (the v1
trace shows loads already stream at that rate); gpsimd 41 x ~1.4us =
~58us is the pacing chain; DVE ~46us hides under both.

Measured v1 baseline: 93.4us.  Rel err ~7e-3 (bf16 inputs + bf16 tree).
"""

import ml_dtypes
import numpy as np

import concourse.bacc as bacc
import concourse.bass as bass
import concourse.mybir as mybir
import concourse.tile as tile
from concourse.bass_utils import run_bass_kernel_spmd

B, H, W = 4, 512, 512
N_PILLARS, N_POINTS, C = 40000, 32, 64
N_CORES = 8
P = 128
HALF = H // 2            # 256 BEV rows per core
REGION_ROWS = HALF * W   # 131072 positions per core
PAD_ROWS = P             # dump rows for padded (inactive) pillars
OUT_ROWS = REGION_ROWS + PAD_ROWS
SUP = 4                  # pillar blocks (of 128) per full super-tile
NBANKS = 8               # independent output tensors breaking scatter WAW chains
BUFS = 6
TAPER = 1                # taper the first/last blocks down to small tiles
BF16 = 1                 # full-bf16 pipeline (host casts; rel err ~7e-3)
TBUFS = 5                # bufs for the small taper tiles


def make_schedule(T, sup=SUP, taper=TAPER):
    """[(base_block, tile_sup), ...] covering blocks 0..T-1.

    Small tiles go FIRST (fast ramp: the first scatter is ready after a
    ~0.5MB load + one short DVE chain instead of a full super-tile) and a
    short taper goes LAST (small end-of-pipe drain)."""
    head = [1, 1, 2, 2] if taper else []
    tail = [2, 1, 1] if taper else []
    extra_blocks = sum(head) + sum(tail)
    while taper and (T - extra_blocks) % sup != 0:
        tail.append(1)
        extra_blocks += 1
    if not taper:
        assert T % sup == 0
    sched = []
    base = 0
    for s in head:
        sched.append((base, s))
        base += s
    for _ in range((T - extra_blocks) // sup):
        sched.append((base, sup))
        base += sup
    for s in tail:
        sched.append((base, s))
        base += s
    assert base == T
    return sched


def build_nc(nmax, n_points=N_POINTS, c=C, out_rows=OUT_ROWS, sup=SUP,
             bufs=BUFS, nbanks=NBANKS, taper=TAPER, bf16=BF16):
    T = nmax // P          # pillar blocks
    D = n_points * c       # full row: 2048 values
    sched = make_schedule(T, sup, taper)
    dt = mybir.dt.bfloat16 if bf16 else mybir.dt.float32
    nc = bacc.Bacc("TRN2", target_bir_lowering=False)
    pf = nc.dram_tensor("pf", [nmax, D], dt, kind="ExternalInput")
    offs = nc.dram_tensor("offs", [P, T], mybir.dt.int32, kind="ExternalInput")
    banks = [
        nc.dram_tensor(f"out{k}", [out_rows, c], dt, kind="ExternalOutput")
        for k in range(nbanks)
    ]
    with tile.TileContext(nc) as tc:
        with (
            tc.tile_pool(name="io", bufs=bufs) as io_pool,
            tc.tile_pool(name="misc", bufs=1) as misc,
        ):
            offs_sb = misc.tile([P, T], mybir.dt.int32)
            sums_sb = misc.tile([P, T * c], dt)
            # scalar engine HW queue: lands ~immediately, not behind the
            # 0.5MB first data tile on the sync queue
            nc.scalar.dma_start(out=offs_sb[:], in_=offs[:])
            for base, s in sched:
                rows = slice(base * P, (base + s) * P)
                sb = io_pool.tile([P, s * D], dt, tag=f"sb{s}",
                                  bufs=bufs if s == sup else TBUFS)
                # pillar j = base*128 + p*s + blk -> partition p; host
                # pre-transposed the group so partition p's contiguous
                # s*D stretch is [q (32), blk (s), c (64)]
                nc.sync.dma_start(
                    out=sb[:],
                    in_=pf[rows, :].rearrange("(p x) w -> p (x w)", x=s),
                )
                # halving-add tree over the leading point axis: 5 full-width
                # contiguous bf16 ops (DVE 2x perf mode); last level lands
                # in the persistent sums buffer
                w = s * D // 2
                while w > s * c:
                    nc.vector.tensor_add(
                        out=sb[:, 0:w], in0=sb[:, 0:w], in1=sb[:, w:2 * w]
                    )
                    w //= 2
                nc.vector.tensor_add(
                    out=sums_sb[:, base * c:(base + s) * c],
                    in0=sb[:, 0:w], in1=sb[:, w:2 * w],
                )
                # one indirect DMA per 128-pillar block (HW: one offset per
                # partition), reading the decoupled sums buffer
                for blk in range(s):
                    g = base + blk
                    nc.gpsimd.indirect_dma_start(
                        out=banks[g % nbanks][:],
                        out_offset=bass.IndirectOffsetOnAxis(
                            ap=offs_sb[:, g:g + 1], axis=0
                        ),
                        in_=sums_sb[:, g * c:(g + 1) * c],
                        in_offset=None,
                    )
    nc.finalize()  # Bacc.compile(): splits multi-waits for TRN2 codegen
    return nc


def shard_inputs(point_features, voxel_coords, sup=SUP, taper=TAPER,
                 bf16=BF16):
    pf = np.ascontiguousarray(
        np.asarray(point_features, dtype=np.float32).reshape(N_PILLARS, N_POINTS * C)
    )
    np_dt = ml_dtypes.bfloat16 if bf16 else np.float32
    vc = np.asarray(voxel_coords)
    b = vc[:, 0].astype(np.int64)
    y = vc[:, 2].astype(np.int64)
    x = vc[:, 3].astype(np.int64)
    upper = (y >= HALF).astype(np.int64)
    region = b * 2 + upper
    off = (y - upper * HALF) * W + x  # row offset within the owned region
    idx_r = [np.nonzero(region == r)[0] for r in range(N_CORES)]
    nmax = max(len(ix) for ix in idx_r)
    nmax = max(P, ((nmax + P - 1) // P) * P)
    if not taper:
        a2 = sup * P
        nmax = ((nmax + a2 - 1) // a2) * a2
    T = nmax // P
    sched = make_schedule(T, sup, taper)
    inv_np = np.float32(1.0 / N_POINTS)
    in_maps = []
    for r in range(N_CORES):
        ix = idx_r[r]
        pf_r = np.zeros((nmax, N_POINTS * C), np_dt)
        # fold the mean's 1/n_points into the gather
        pf_r[: len(ix)] = (pf[ix] * inv_np).astype(np_dt)
        offs_r = np.full(nmax, REGION_ROWS, np.int32)  # pad rows -> dump row
        offs_r[: len(ix)] = off[ix].astype(np.int32)
        # tile (base, s): pillar j = base*128 + p*s + blk -> offs_arr[p, base+blk]
        offs_arr = np.empty((P, T), np.int32)
        for base, s in sched:
            seg = offs_r[base * P:(base + s) * P].reshape(P, s)
            offs_arr[:, base:base + s] = seg
            # transpose partition p's s pillars from [blk, q, c] to
            # [q, blk, c] so the on-device add tree is contiguous
            blockrows = pf_r[base * P:(base + s) * P]
            t = blockrows.reshape(P, s, N_POINTS, C).transpose(0, 2, 1, 3)
            pf_r[base * P:(base + s) * P] = t.reshape(P * s, N_POINTS * C)
        in_maps.append({"pf": pf_r, "offs": np.ascontiguousarray(offs_arr)})
    return in_maps, nmax


def assemble(results):
    out = np.empty((B, C, H, W), np.float32)
    for r in range(N_CORES):
        names = sorted(results[r])       # out0..out{nbanks-1}
        region = np.asarray(results[r][names[0]], dtype=np.float32)
        for name in names[1:]:
            # banks: disjoint row support
            region = region + np.asarray(results[r][name], dtype=np.float32)
        o = region[:REGION_ROWS].reshape(HALF, W, C)
        b_, half = divmod(r, 2)
        out[b_, :, half * HALF:(half + 1) * HALF, :] = o.transpose(2, 0, 1)
    return out


def run(point_features, voxel_coords, trace=False, sup=SUP, bufs=BUFS,
        nbanks=NBANKS, taper=TAPER, bf16=BF16, **spmd_kwargs):
    in_maps, nmax = shard_inputs(point_features, voxel_coords,
                                 sup=sup, taper=taper, bf16=bf16)
    nc = build_nc(nmax, sup=sup, bufs=bufs, nbanks=nbanks, taper=taper,
                  bf16=bf16)
    br = run_bass_kernel_spmd(
        nc, in_maps, list(range(N_CORES)), trace=trace, **spmd_kwargs
    )
    return assemble(br.results), br


def kernel(point_features, voxel_coords):
    out, _ = run(point_features, voxel_coords)
    return out
